# revision 45
# baseline (speedup 1.0000x reference)
# Causal self-attention on 8 TRN2 NeuronCores.
#
# Sharding (data + tensor parallel per the hint):
#   core c -> batch b = c // 4, head group g = c % 4 (4 heads of 64 dims = 256).
#   Wq/Wk/Wv split column-wise per head group; Wo row-wise. Each core computes
#   a partial [D, S] output; the host sums the 4 partials per batch element
#   (the "all-reduce" of row-parallel sharding) and transposes back.
#
# Optimizations vs the 237us fp32r baseline (now ~170-178us, rel err 4e-3):
#   - All matmul operands bf16 (PE streams 1 row/cycle for bf16 and f32r
#     alike, but bf16 halves the input/output DMA and SBUF traffic).
#   - Off-diagonal PV uses fp8e4m3 DoubleRow (2 k-chunks contracted per
#     pass, 0.5 cyc/row); the diagonal chunks stay bf16 because few-key
#     softmax rows near the diagonal cannot average away fp8 quantization
#     noise (fp8 there costs 3e-2 rel err; off-diag fp8 costs nothing).
#     exp is biased by EXPB so pt fits fp8's +-448 range; the scale cancels
#     in the softmax normalization exactly.
#   - No giant PE warmup: 14 dummy matmuls (~3us) ramp the p-state while
#     the input DMA lands.
#   - q/k biases folded into the PSUM->SBUF copy on DVE (per-partition
#     broadcast); v bias and bo folded into a host-side bias after the
#     gather (softmax rows sum to 1); no bias matmuls on PE.
#   - Causal mask applied on the PE as an extra accumulation matmul
#     (identity x (-300 * mask)) before exp, removing the DVE mask hop from
#     the scores -> exp -> PV critical chain.
#   - PV for chunk-pair cp is emitted after scores for pair cp+1
#     (lookahead-1) with proj/out-proj fill thunks in between to hide the
#     ACT exp latency. Fill thunks must only reference data already in
#     SBUF or the in-order PE stalls on them.
#   - DMA triggers for the bulk of x stay off the Scalar engine queue where
#     possible: a trigger waiting for a DMA queue slot blocks every exp
#     emitted after it on that engine.
#   - Normalization: rowsums (ones column in V) -> PE broadcast -> DVE
#     reciprocal straight from PSUM; the A-half O is copied out of PSUM
#     early so the next block's PV can reuse the accumulation bank.

import os

import numpy as np

S = 2048
D = 1024
DL = 256  # local head dims (4 heads x 64)
NCORES = 8

_cache = {}
LAST_EXEC_TIME_NS = None
LAST_TRACE_PATH = None


def _build_bass():
    from concourse import bacc
    import concourse.tile as tile
    import concourse.mybir as mybir
    from concourse.bass import ts, ds

    f32 = mybir.dt.float32
    f32r = mybir.dt.float32r
    bf16 = mybir.dt.bfloat16
    fp8 = mybir.dt.float8e4
    DR = mybir.MatmulPerfMode.DoubleRow
    Exp = mybir.ActivationFunctionType.Exp
    ADD = mybir.AluOpType.add
    # exp(score - EXPB) keeps pt within fp8e4m3 range; the scale cancels in
    # the softmax normalization exactly
    EXPB = -4.15888

    nc = bacc.Bacc("TRN2", target_bir_lowering=False, debug=False)

    xT_d = nc.dram_tensor("xT", [D, S], bf16, kind="ExternalInput")
    wqT_d = nc.dram_tensor("wqT", [D, DL], bf16, kind="ExternalInput")
    wkT_d = nc.dram_tensor("wkT", [D, DL], bf16, kind="ExternalInput")
    wvT_d = nc.dram_tensor("wvT", [D, DL], bf16, kind="ExternalInput")
    woT_d = nc.dram_tensor("woT", [DL, D], bf16, kind="ExternalInput")
    bqs_d = nc.dram_tensor("bqs", [128, 2], f32r, kind="ExternalInput")
    bks_d = nc.dram_tensor("bks", [128, 2], f32r, kind="ExternalInput")
    maskneg_d = nc.dram_tensor("maskneg", [128, 256], bf16, kind="ExternalInput")
    ident_d = nc.dram_tensor("ident", [128, 128], bf16, kind="ExternalInput")
    onesf_d = nc.dram_tensor("onesf", [128, 64], f32r, kind="ExternalInput")
    onesv_d = nc.dram_tensor("onesv", [128, 8, 4, 2, 1], fp8, kind="ExternalInput")
    onesv2_d = nc.dram_tensor("onesv2", [128, 16, 4, 1], bf16, kind="ExternalInput")
    out_d = nc.dram_tensor("outT", [D, S], bf16, kind="ExternalOutput")
    # out_d receives ps (f32 PSUM) casts via gpsimd-initiated DMAs
    warm_d = nc.dram_tensor("warm", [1, 514], f32, kind="ExternalOutput")

    with tile.TileContext(nc) as tc:
        with (
            tc.tile_pool(name="persist", bufs=1) as persist,
            tc.tile_pool(name="ptp", bufs=4) as ptp,
            tc.tile_pool(name="sup", bufs=2) as sup,
            tc.tile_pool(name="rbp", bufs=2) as rbp,
            tc.tile_pool(name="stp", bufs=2) as stp,
            tc.tile_pool(name="tbp", bufs=2) as tbp,
            tc.tile_pool(name="sc2", bufs=2, space="PSUM") as sc2,
            tc.tile_pool(name="mm", bufs=2, space="PSUM") as mm,
            tc.tile_pool(name="po", bufs=2, space="PSUM") as po,
        ):
            # ---- persistent SBUF tensors ----
            xT = persist.tile([128, 8, S], bf16, name="xT_sb")
            wqT = persist.tile([128, 8, DL], bf16, name="wqT_sb")
            wkT = persist.tile([128, 8, DL], bf16, name="wkT_sb")
            wvT = persist.tile([128, 8, DL], bf16, name="wvT_sb")
            woT = persist.tile([128, 2, D], bf16, name="woT_sb")
            bqs = persist.tile([128, 2], f32r, name="bqs_sb")
            bks = persist.tile([128, 2], f32r, name="bks_sb")
            maskneg = persist.tile([128, 2, 128], bf16, name="maskneg_sb")
            ident = persist.tile([128, 128], bf16, name="ident_sb")
            onesf = persist.tile([128, 64], f32r, name="onesf_sb")
            wbuf = persist.tile([128, 512], bf16, name="wbuf_sb")
            qT = persist.tile([128, 2, S], bf16, name="qT_sb")
            kT = persist.tile([128, 2, S], bf16, name="kT_sb")
            # v8: [s-part, chunk-pair, head, chunk-parity, 64 v dims + ones]
            # fp8 copy feeds off-diagonal DoubleRow PV; bf16 copy (v4b) feeds
            # the diagonal chunks where few-key softmax rows can't average
            # away fp8 quantization noise
            v8 = persist.tile([128, 8, 4, 2, 80], fp8, name="v8_sb")
            v4b = persist.tile([128, 16, 4, 65], bf16, name="v4b_sb")
            oT = persist.tile([128, 2, S], bf16, name="oT_sb")

            # ---- memsets (no DMA dependency) ----
            nc.vector.memset(wbuf[:], 1.0)
            expb = persist.tile([128, 1], f32, name="expb_sb")
            nc.gpsimd.memset(expb[:], EXPB)

            # ---- input DMAs ----
            wq_r = wqT_d.ap().rearrange("(o p) f -> p o f", p=128)
            wk_r = wkT_d.ap().rearrange("(o p) f -> p o f", p=128)
            wv_r = wvT_d.ap().rearrange("(o p) f -> p o f", p=128)
            wo_r = woT_d.ap().rearrange("(o p) f -> p o f", p=128)
            x_r = xT_d.ap().rearrange("(o p) f -> p o f", p=128)
            # sync queue: wq, x tb0 evens, small tensors, then rest evens
            # scalar queue: wk, x tb0 odds, wv, wo, then rest odds
            nc.sync.dma_start(wqT[:], wq_r)
            nc.scalar.dma_start(wkT[:], wk_r)
            for mc in range(0, 8, 2):
                nc.sync.dma_start(xT[:, mc, 0:512], x_r[:, mc, 0:512])
            for mc in range(1, 8, 2):
                nc.scalar.dma_start(xT[:, mc, 0:512], x_r[:, mc, 0:512])
            nc.sync.dma_start(bqs[:], bqs_d.ap())
            nc.sync.dma_start(bks[:], bks_d.ap())
            nc.sync.dma_start(
                maskneg[:], maskneg_d.ap().rearrange("p (h q) -> p h q", h=2)
            )
            nc.sync.dma_start(ident[:], ident_d.ap())
            nc.sync.dma_start(onesf[:], onesf_d.ap())
            nc.sync.dma_start(v8[:, :, :, :, 64:65], onesv_d.ap())
            nc.sync.dma_start(v4b[:, :, :, 64:65], onesv2_d.ap())
            nc.scalar.dma_start(wvT[:], wv_r)
            nc.scalar.dma_start(woT[:], wo_r)
            for tb in range(1, 4):
                for mc in range(8):
                    eng = nc.sync if mc % 2 == 0 else nc.scalar
                    eng.dma_start(
                        xT[:, mc, ts(tb, 512)], x_r[:, mc, ts(tb, 512)]
                    )

            # ---- small PE warmup (p-state ramp while DMA lands) + ACT warm
            psW = mm.tile([128, 512], f32, tag="mm", name="psW")
            for _ in range(14):
                nc.tensor.matmul(
                    psW,
                    lhsT=wbuf[:, 0:128],
                    rhs=wbuf[:],
                    start=True,
                    stop=True,
                    skip_group_check=True,
                )
            wstg = stp.tile([1, 514], f32, tag="wst", name="wstg", bufs=1)
            nc.vector.tensor_copy(wstg[0:1, 0:512], psW[0:1, :])
            nc.scalar.activation(wstg[0:1, 512:514], wbuf[0:1, 0:2], Exp)
            nc.sync.dma_start(warm_d.ap(), wstg[:])

            def proj_qk(wsb, bsb, dst, t, qb):
                ps = mm.tile([128, 512], f32, tag="mm")
                for mc in range(8):
                    nc.tensor.matmul(
                        ps,
                        lhsT=wsb[:, mc, ts(t, 128)],
                        rhs=xT[:, mc, ts(qb, 512)],
                        start=(mc == 0),
                        stop=(mc == 7),
                    )
                nc.vector.tensor_tensor(
                    dst[:, t, ts(qb, 512)],
                    ps,
                    bsb[:, t : t + 1].to_broadcast((128, 512)),
                    ADD,
                )

            def proj_v(st):
                ps = mm.tile([128, 512], f32, tag="mm")
                psv = ps[:, 0:256]
                for mc in range(8):
                    nc.tensor.matmul(
                        psv,
                        lhsT=xT[:, mc, ts(st, 128)],
                        rhs=wvT[:, mc, :],
                        start=(mc == 0),
                        stop=(mc == 7),
                    )
                nc.vector.tensor_copy(
                    v8[:, st // 2, :, st % 2, 0:64],
                    psv.rearrange("p (h d) -> p h d", h=4),
                )
                nc.vector.tensor_copy(
                    v4b[:, st, :, 0:64], psv.rearrange("p (h d) -> p h d", h=4)
                )

            def attn_block(pair, qb, fill=None, fill_every=1):
                # heads (2*pair, 2*pair+1); q columns [512*qb, 512*qb+512)
                psA = po.tile([128, 512], f32, tag="po")
                psB = po.tile([128, 512], f32, tag="po")
                npairs = 2 * qb + 2
                prev = None  # pending PV for chunk-pair cp (lookahead-1)

                def emit_pv(p):
                    pt, cp, offd, ws = p
                    if offd:
                        for hh, psO in ((0, psA), (1, psB)):
                            nc.tensor.matmul(
                                psO[0:65, :],
                                lhsT=v8[:, cp, 2 * pair + hh, :, 0:65],
                                rhs=pt[:, hh, :, :],
                                perf_mode=DR,
                                start=(cp == 0),
                                stop=False,
                                skip_group_check=True,
                            )
                    else:
                        for sub in (0, 1):
                            q0, w = ws[sub]
                            for hh, psO in ((0, psA), (1, psB)):
                                nc.tensor.matmul(
                                    psO[0:65, ds(q0, w)],
                                    lhsT=v4b[:, 2 * cp + sub, 2 * pair + hh, :],
                                    rhs=pt[:, hh, sub, :w],
                                    start=(cp == 0 and sub == 0),
                                    stop=(cp == npairs - 1 and sub == 1),
                                    skip_group_check=True,
                                )

                for cp in range(npairs):
                    offd = cp < 2 * qb
                    if offd:
                        pt = ptp.tile([128, 2, 2, 512], fp8, tag="pt")
                    else:
                        pt = ptp.tile([128, 2, 2, 512], bf16, tag="ptb", bufs=2)
                    ws = []
                    for sub in (0, 1):
                        c = 2 * cp + sub
                        dc = c - 4 * qb
                        diag = dc >= 0
                        q0 = 128 * dc if diag else 0
                        w = 512 - q0
                        ws.append((q0, w))
                        ps2 = sc2.tile([128, 2, 512], f32, tag="sc")
                        for hh in (0, 1):
                            prow = slice(64 * hh, 64 * hh + 64)
                            nc.tensor.matmul(
                                ps2[:, hh, :w],
                                lhsT=kT[prow, pair, ts(c, 128)],
                                rhs=qT[prow, pair, ds(512 * qb + q0, w)],
                                start=True,
                                stop=(not diag),
                                skip_group_check=True,
                            )
                        if diag:
                            nc.tensor.matmul(
                                ps2[:, :, 0:128],
                                lhsT=ident[:],
                                rhs=maskneg[:],
                                start=False,
                                stop=True,
                                skip_group_check=True,
                            )
                        nc.scalar.activation(
                            pt[:, :, sub, :w], ps2[:, :, :w], Exp, bias=expb[:]
                        )
                    if fill and cp % fill_every == fill_every - 1:
                        fill.pop(0)()
                    if prev is not None:
                        emit_pv(prev)
                    prev = (pt, cp, offd, ws)
                emit_pv(prev)

                # normalization: rowsums -> SBUF -> PE broadcast -> DVE
                # reciprocal (PSUM->SBUF) -> DVE multiply. The A-half O is
                # copied out of PSUM immediately so psA frees before the
                # 3-hop reciprocal chain (the next block's PV reuses the buf)
                sums = sup.tile([65, 1024], f32r, tag="su")
                nc.vector.tensor_copy(sums[64:65, 0:512], psA[64:65, :])
                nc.vector.tensor_copy(sums[64:65, 512:1024], psB[64:65, :])
                oUA = tbp.tile([64, 512], f32, tag="ou", bufs=2)
                nc.vector.tensor_copy(oUA[:, :], psA[0:64, :])
                psRA = mm.tile([128, 512], f32, tag="mm")
                nc.tensor.matmul(
                    psRA[0:64, :],
                    lhsT=onesf[64:65, 0:64],
                    rhs=sums[64:65, 0:512],
                    start=True,
                    stop=True,
                )
                rbA = rbp.tile([64, 512], f32, tag="rb")
                nc.vector.reciprocal_approx_fast(rbA[:, :], psRA[0:64, :])
                nc.vector.tensor_mul(
                    oT[0:64, pair, ts(qb, 512)], oUA[:, :], rbA[:, :]
                )
                psRB = mm.tile([128, 512], f32, tag="mm")
                nc.tensor.matmul(
                    psRB[0:64, :],
                    lhsT=onesf[64:65, 0:64],
                    rhs=sums[64:65, 512:1024],
                    start=True,
                    stop=True,
                )
                rbB = rbp.tile([64, 512], f32, tag="rb")
                nc.vector.reciprocal_approx_fast(rbB[:, :], psRB[0:64, :])
                tmpB = tbp.tile([64, 512], bf16, tag="tb")
                nc.vector.tensor_mul(tmpB[:, :], psB[0:64, :], rbB[:, :])
                nc.gpsimd.dma_start(oT[64:128, pair, ts(qb, 512)], tmpB[:, :])

            def out_proj_jt(jt, sb, use_act=False):
                # DoubleRow: both 128-dim d' subtiles contracted in one pass;
                # bias bo is applied on the host after the partial gather
                ps = mm.tile([128, 512], f32, tag="mm")
                for dchunk in range(2):
                    nc.tensor.matmul(
                        ps,
                        lhsT=woT[:, dchunk, ts(jt, 128)],
                        rhs=oT[:, dchunk, ts(sb, 512)],
                        start=(dchunk == 0),
                        stop=(dchunk == 1),
                    )
                stg = stp.tile([128, 512], bf16, tag="st")
                if use_act:
                    nc.scalar.copy(stg[:], ps)
                else:
                    nc.vector.tensor_copy(stg[:], ps)
                nc.sync.dma_start(out_d.ap()[ts(jt, 128), ts(sb, 512)], stg[:])

            # software-pipelined emission: per q-block wave, produce the
            # projections it needs, then attention, then the output slice
            def emit_A(qb):
                for t in range(2):
                    proj_qk(wqT, bqs, qT, t, qb)
                for st in range(4 * qb, 4 * qb + 4):
                    proj_v(st)
                for t in range(2):
                    proj_qk(wkT, bks, kT, t, qb)

            emit_A(0)
            for qb in range(4):
                ath = []
                if qb < 3:
                    nxt = qb + 1
                    for t in range(2):
                        ath.append(
                            lambda t=t, nxt=nxt: proj_qk(wqT, bqs, qT, t, nxt)
                        )
                    for st in range(4 * nxt, 4 * nxt + 4):
                        ath.append(lambda st=st: proj_v(st))
                    for t in range(2):
                        ath.append(
                            lambda t=t, nxt=nxt: proj_qk(wkT, bks, kT, t, nxt)
                        )
                cth = []
                if qb == 1:
                    cth = [
                        lambda jt=jt: out_proj_jt(jt, 0) for jt in range(8)
                    ]
                elif qb == 2:
                    cth = [
                        lambda jt=jt: out_proj_jt(jt, 1) for jt in range(4)
                    ]
                elif qb == 3:
                    cth = [
                        lambda jt=jt: out_proj_jt(jt + 4, 1) for jt in range(4)
                    ] + [
                        lambda jt=jt: out_proj_jt(jt, 2) for jt in range(8)
                    ]
                thunks = []
                for i in range(max(len(ath), len(cth))):
                    if i < len(ath):
                        thunks.append(ath[i])
                    if i < len(cth):
                        thunks.append(cth[i])
                fe = max(1, (2 * (2 * qb + 2)) // (len(thunks) + 1))
                attn_block(0, qb, fill=thunks, fill_every=fe)
                attn_block(1, qb, fill=thunks, fill_every=fe)
                for th in thunks:
                    th()
            for jt in range(8):
                out_proj_jt(jt, 3, use_act=(jt % 2 == 1))

    nc.compile()
    return nc


def _get_bass():
    if "nc" not in _cache:
        _cache["nc"] = _build_bass()
    return _cache["nc"]


def _shard_inputs(x, Wq, bq, Wk, bk, Wv, bv, Wo, bo):
    import ml_dtypes

    bf16 = ml_dtypes.bfloat16
    fp8 = ml_dtypes.float8_e4m3fn
    x = np.asarray(x, dtype=np.float32)
    Wq = np.asarray(Wq, dtype=np.float32)
    Wk = np.asarray(Wk, dtype=np.float32)
    Wv = np.asarray(Wv, dtype=np.float32)
    Wo = np.asarray(Wo, dtype=np.float32)
    bq = np.asarray(bq, dtype=np.float32)
    bk = np.asarray(bk, dtype=np.float32)
    bv = np.asarray(bv, dtype=np.float32)
    bo = np.asarray(bo, dtype=np.float32)

    kk = np.arange(128)[:, None]
    qq = np.arange(128)[None, :]
    mask1 = np.where(kk > qq, np.float32(-300.0), np.float32(0.0))
    maskneg = np.ascontiguousarray(
        np.concatenate([mask1, mask1], axis=1)
    ).astype(bf16)
    ident = np.eye(128, dtype=np.float32).astype(bf16)

    xT = [np.ascontiguousarray(x[b].T).astype(bf16) for b in range(x.shape[0])]
    in_maps = []
    for c in range(NCORES):
        b, g = divmod(c, 4)
        sl = slice(DL * g, DL * (g + 1))
        in_maps.append(
            {
                "xT": xT[b],
                "wqT": (np.ascontiguousarray(Wq[sl].T) * 0.125).astype(bf16),
                "wkT": np.ascontiguousarray(Wk[sl].T).astype(bf16),
                "wvT": np.ascontiguousarray(Wv[sl].T).astype(bf16),
                "woT": np.ascontiguousarray(Wo[:, sl].T).astype(bf16),
                "bqs": np.ascontiguousarray(
                    (bq[sl] * 0.125).reshape(2, 128).T
                ),
                "bks": np.ascontiguousarray(bk[sl].reshape(2, 128).T),
                "maskneg": maskneg,
                "ident": ident,
                "onesf": np.ones((128, 64), np.float32),
                "onesv": np.ones((128, 8, 4, 2, 1), np.float32).astype(fp8),
                "onesv2": np.ones((128, 16, 4, 1), np.float32).astype(bf16),
            }
        )
    return in_maps


def kernel(x, Wq, bq, Wk, bk, Wv, bv, Wo, bo):
    global LAST_EXEC_TIME_NS, LAST_TRACE_PATH
    from concourse.bass_utils import run_bass_kernel_spmd

    nc = _get_bass()
    in_maps = _shard_inputs(x, Wq, bq, Wk, bk, Wv, bv, Wo, bo)

    trace = os.environ.get("KERNEL_TRACE", "0") == "1"
    res = run_bass_kernel_spmd(
        nc, in_maps, core_ids=list(range(NCORES)), trace=trace
    )
    LAST_EXEC_TIME_NS = res.exec_time_ns
    if res.instructions_and_trace is not None:
        LAST_TRACE_PATH = res.instructions_and_trace[1]

    # bo and the folded v-bias contribution (softmax rows sum to 1, so bv
    # commutes through attention into the output projection) on the host
    bias = (
        np.asarray(bo, np.float32)
        + np.asarray(Wo, np.float32) @ np.asarray(bv, np.float32)
    )
    B = 2
    out = np.empty((B, S, D), dtype=np.float32)
    for b in range(B):
        acc = res.results[4 * b]["outT"].astype(np.float32)
        for g in range(1, 4):
            acc = acc + res.results[4 * b + g]["outT"].astype(np.float32)
        out[b] = acc.T + bias
    return out
